# revision 1
# baseline (speedup 1.0000x reference)
"""Context2Query kernel for Trainium2 (Bass/Tile), 8 NeuronCores.

Computes, for inputs u[B, M, D] (query encodings) and s[B, N, M]
(similarity logits):

    A = softmax(s, axis=-1)            # [B, N, M]
    out = einsum('bnm,bmd->bdn', A, u) # [B, D, N]

Sharding: data-parallel over batch. B=16 across 8 cores -> 2 batches/core.
Per batch, per 128-row tile of s (n on partitions):
  - ACT: E = exp(s) in bf16 with fused row-sum (accum_out)  [no max-sub
    needed: logits are N(0,1), exp range ~e^+-6 is safe in fp32]
  - DVE: rinv = 1/sum;  A = E * rinv (per-partition scalar)
  - DMA xbar transpose (SBUF->SBUF, bf16): A tile -> A^T laid out
    [m_in_block(P), t, m_block, n] so the tensor engine sees contraction
    dim m on partitions.
  - PE: out[dblk, n-chunk] += u_bf16[mblk, dblk]^T @ A^T[mblk, n-chunk]
    accumulated over 16 m-blocks in one PSUM bank. PE does only matmuls
    (stays HAM-warm).
u is cast fp32->bf16 for free during its DMA load (SWDGE cast).
"""

import numpy as np

import concourse.bacc as bacc
import concourse.mybir as mybir
import concourse.tile as tile
from concourse.bass_utils import run_bass_kernel_spmd
from concourse.masks import make_identity

F32 = mybir.dt.float32
BF16 = mybir.dt.bfloat16
P = 128

N_CORES = 8


def build_nc(
    B_local,
    N,
    M,
    D,
    n_cores=N_CORES,
    NT=512,
    repeat=1,
    tr_mode="dma",
    db_lim=None,
    u_bf16_host=False,
    skip_tr=False,
    dep_free=False,
    tr_eng="sync",
    mix_pe_ts=(1, 3),
    tr_grouped=False,
    rhs_contig=False,
    sim_trace=False,
):
    assert N % NT == 0 and M % P == 0 and D % P == 0 and NT % P == 0
    assert tr_mode in ("dma", "pe", "mix", "pe2", "hy")
    nc = bacc.Bacc("TRN2", target_bir_lowering=False, num_devices=n_cores)
    s = nc.dram_tensor("s", [B_local, N, M], F32, kind="ExternalInput").ap()
    u_dt = BF16 if u_bf16_host else F32
    u = nc.dram_tensor("u", [B_local, M, D], u_dt, kind="ExternalInput").ap()
    out = nc.dram_tensor("out", [B_local, D, N], F32, kind="ExternalOutput").ap()

    MB = M // P  # contraction blocks
    DB = D // P  # output-partition blocks
    NCH = N // NT  # n chunks
    T = NT // P  # 128-row subtiles per chunk

    with tile.TileContext(nc, trace_sim=sim_trace) as tc:
        with (
            tc.tile_pool(name="u_pool", bufs=2) as u_pool,
            tc.tile_pool(name="s_pool", bufs=4) as s_pool,
            tc.tile_pool(
                name="e_pool",
                bufs=(6 if (tr_grouped or tr_mode in ("pe2", "hy")) else 3),
            ) as e_pool,
            tc.tile_pool(name="at_pool", bufs=2) as at_pool,
            tc.tile_pool(name="o_pool", bufs=2) as o_pool,
            tc.tile_pool(name="st_pool", bufs=4 * T) as st_pool,
            tc.tile_pool(name="singles", bufs=1) as singles,
            tc.tile_pool(name="ps_pool", bufs=4, space="PSUM") as ps_pool,
            tc.tile_pool(name="pst_pool", bufs=2, space="PSUM") as pst_pool,
        ):
            identity = None
            if tr_mode in ("pe", "mix", "pe2", "hy"):
                identity = singles.tile([P, P], BF16)
                make_identity(nc, identity)
            at_const = None
            if dep_free:
                at_const = singles.tile([P, T, MB, P], BF16)
                nc.vector.memset(at_const[:], 0)
            tr_dma = nc.sync if tr_eng == "sync" else nc.scalar
            for b in [b for _ in range(repeat) for b in range(B_local)]:
                u_bf = u_pool.tile([P, MB, D], BF16)
                u_src = u[b].rearrange("(mB p) d -> p mB d", p=P)
                if u_bf16_host:
                    nc.sync.dma_start(out=u_bf[:], in_=u_src)
                else:
                    nc.gpsimd.dma_start(out=u_bf[:], in_=u_src)
                if tr_mode == "hy":
                    # Hybrid pipeline: per chunk, subtiles 0/2 transpose via
                    # DMA xbar (pair issued adjacently to amortize the
                    # xbar-mode switch), subtiles 1/3 via PE in small packets
                    # interleaved between matmul groups.
                    GRP = MB // 2
                    e_cur = {}

                    def produce_e(cc, t):
                        n0 = cc * NT + t * P
                        s_t = s_pool.tile([P, M], F32)
                        nc.sync.dma_start(out=s_t[:], in_=s[b, n0 : n0 + P, :])
                        e_t = e_pool.tile([P, M], BF16)
                        sum_t = st_pool.tile([P, 1], F32, tag="sum")
                        nc.scalar.activation(
                            out=e_t[:],
                            in_=s_t[:],
                            func=mybir.ActivationFunctionType.Exp,
                            accum_out=sum_t[:],
                        )
                        rinv = st_pool.tile([P, 1], F32, tag="rinv")
                        nc.vector.reciprocal(rinv[:], sum_t[:])
                        nc.vector.tensor_scalar_mul(e_t[:], e_t[:], rinv[:])
                        e_cur[(cc, t)] = e_t

                    def pe_tr_half(cc, t, half, AT_next):
                        e_t = e_cur[(cc, t)]
                        ps_t = pst_pool.tile([P, GRP, P], BF16)
                        for k in range(GRP):
                            mblk = half * GRP + k
                            nc.tensor.transpose(
                                ps_t[:, k], e_t[:, mblk * P : (mblk + 1) * P], identity
                            )
                        at_dst = AT_next[:, t, half * GRP : (half + 1) * GRP, :]
                        if half == 0:
                            nc.vector.tensor_copy(out=at_dst, in_=ps_t[:])
                        else:
                            nc.scalar.copy(out=at_dst, in_=ps_t[:])

                    def produce_slot(cc, slot, AT_next):
                        if slot == 0:
                            produce_e(cc, 0)
                        elif slot == 1:
                            produce_e(cc, 1)
                            pe_tr_half(cc, 1, 0, AT_next)
                        elif slot == 2:
                            pe_tr_half(cc, 1, 1, AT_next)
                        elif slot == 3:
                            produce_e(cc, 2)
                            nc.sync.dma_start_transpose(
                                AT_next[:, 0], e_cur[(cc, 0)][:]
                            )
                            nc.sync.dma_start_transpose(
                                AT_next[:, 2], e_cur[(cc, 2)][:]
                            )
                        elif slot == 4:
                            produce_e(cc, 3)
                            pe_tr_half(cc, 3, 0, AT_next)
                        elif slot == 5:
                            pe_tr_half(cc, 3, 1, AT_next)

                    NSLOT = 6
                    ATs = at_pool.tile([P, T, MB, P], BF16)
                    for slot in range(NSLOT):
                        produce_slot(0, slot, ATs)
                    for c in range(NCH):
                        AT_cur = ATs
                        if c + 1 < NCH:
                            ATs = at_pool.tile([P, T, MB, P], BF16)
                        o_t = o_pool.tile([P, DB, NT], F32)
                        for dblk in range(DB):
                            ps = ps_pool.tile([P, NT], F32)
                            for mblk in range(MB):
                                nc.tensor.matmul(
                                    ps[:],
                                    u_bf[:, mblk, dblk * P : (dblk + 1) * P],
                                    AT_cur[:, :, mblk, :],
                                    start=(mblk == 0),
                                    stop=(mblk == MB - 1),
                                )
                            if dblk % 2 == 0:
                                nc.vector.tensor_copy(out=o_t[:, dblk, :], in_=ps[:])
                            else:
                                nc.scalar.copy(out=o_t[:, dblk, :], in_=ps[:])
                            if c + 1 < NCH and dblk < NSLOT:
                                produce_slot(c + 1, dblk, ATs)
                        if c + 1 < NCH:
                            for slot in range(DB, NSLOT):
                                produce_slot(c + 1, slot, ATs)
                        nc.sync.dma_start(
                            out=out[b].rearrange("(dB p) n -> p dB n", p=P)[
                                :, :, c * NT : (c + 1) * NT
                            ],
                            in_=o_t[:],
                        )
                    continue
                if tr_mode == "pe2":
                    # Software-pipelined emission: chunk c+1's softmax +
                    # PE-transposes are emitted in small packets between
                    # chunk c's matmul groups, so PE transpose bursts stay
                    # short (HAM stays warm) and overlap naturally.
                    GRP = MB // 2
                    e_cur = {}

                    def produce_packet(cc, pkt, AT_next):
                        t, half = pkt // 2, pkt % 2
                        if half == 0:
                            n0 = cc * NT + t * P
                            s_t = s_pool.tile([P, M], F32)
                            nc.sync.dma_start(out=s_t[:], in_=s[b, n0 : n0 + P, :])
                            e_t = e_pool.tile([P, M], BF16)
                            sum_t = st_pool.tile([P, 1], F32, tag="sum")
                            nc.scalar.activation(
                                out=e_t[:],
                                in_=s_t[:],
                                func=mybir.ActivationFunctionType.Exp,
                                accum_out=sum_t[:],
                            )
                            rinv = st_pool.tile([P, 1], F32, tag="rinv")
                            nc.vector.reciprocal(rinv[:], sum_t[:])
                            nc.vector.tensor_scalar_mul(e_t[:], e_t[:], rinv[:])
                            e_cur[(cc, t)] = e_t
                        e_t = e_cur[(cc, t)]
                        ps_t = pst_pool.tile([P, GRP, P], BF16)
                        for k in range(GRP):
                            mblk = half * GRP + k
                            nc.tensor.transpose(
                                ps_t[:, k], e_t[:, mblk * P : (mblk + 1) * P], identity
                            )
                        at_dst = AT_next[:, t, half * GRP : (half + 1) * GRP, :]
                        if pkt % 2 == 0:
                            nc.vector.tensor_copy(out=at_dst, in_=ps_t[:])
                        else:
                            nc.scalar.copy(out=at_dst, in_=ps_t[:])

                    ATs = at_pool.tile([P, T, MB, P], BF16)
                    for pkt in range(2 * T):
                        produce_packet(0, pkt, ATs)
                    for c in range(NCH):
                        AT_cur = ATs
                        if c + 1 < NCH:
                            ATs = at_pool.tile([P, T, MB, P], BF16)
                        o_t = o_pool.tile([P, DB, NT], F32)
                        for dblk in range(DB):
                            ps = ps_pool.tile([P, NT], F32)
                            for mblk in range(MB):
                                nc.tensor.matmul(
                                    ps[:],
                                    u_bf[:, mblk, dblk * P : (dblk + 1) * P],
                                    AT_cur[:, :, mblk, :],
                                    start=(mblk == 0),
                                    stop=(mblk == MB - 1),
                                )
                            if dblk % 2 == 0:
                                nc.vector.tensor_copy(out=o_t[:, dblk, :], in_=ps[:])
                            else:
                                nc.scalar.copy(out=o_t[:, dblk, :], in_=ps[:])
                            if c + 1 < NCH and dblk < 2 * T:
                                produce_packet(c + 1, dblk, ATs)
                        if c + 1 < NCH:
                            for pkt in range(DB, 2 * T):
                                produce_packet(c + 1, pkt, ATs)
                        nc.sync.dma_start(
                            out=out[b].rearrange("(dB p) n -> p dB n", p=P)[
                                :, :, c * NT : (c + 1) * NT
                            ],
                            in_=o_t[:],
                        )
                    continue
                for c in range(NCH):
                    AT = at_pool.tile([P, T, MB, P], BF16)
                    if skip_tr:
                        nc.vector.memset(AT[:], 0)
                    e_tiles = {}
                    for t in range(T):
                        if skip_tr:
                            break
                        n0 = c * NT + t * P
                        s_t = s_pool.tile([P, M], F32)
                        nc.sync.dma_start(out=s_t[:], in_=s[b, n0 : n0 + P, :])
                        e_t = e_pool.tile([P, M], BF16)
                        sum_t = st_pool.tile([P, 1], F32, tag="sum")
                        nc.scalar.activation(
                            out=e_t[:],
                            in_=s_t[:],
                            func=mybir.ActivationFunctionType.Exp,
                            accum_out=sum_t[:],
                        )
                        rinv = st_pool.tile([P, 1], F32, tag="rinv")
                        nc.vector.reciprocal(rinv[:], sum_t[:])
                        nc.vector.tensor_scalar_mul(e_t[:], e_t[:], rinv[:])
                        if tr_grouped:
                            e_tiles[t] = e_t
                            continue
                        _emit_transpose(
                            nc, tr_mode, tr_dma, mix_pe_ts, AT, t, e_t,
                            identity, pst_pool, MB,
                        )
                    if tr_grouped and not skip_tr:
                        for t in range(T):
                            _emit_transpose(
                                nc, tr_mode, tr_dma, mix_pe_ts, AT, t,
                                e_tiles[t], identity, pst_pool, MB,
                            )
                    o_t = o_pool.tile([P, DB, NT], F32)
                    for dblk in range(DB if db_lim is None else db_lim):
                        ps = ps_pool.tile([P, NT], F32)
                        rhs_src = at_const if dep_free else AT
                        for mblk in range(MB):
                            if rhs_contig:
                                rhs = rhs_src[:, mblk % T, 0 : NT // P, :]
                            else:
                                rhs = rhs_src[:, :, mblk, :]
                            nc.tensor.matmul(
                                ps[:],
                                u_bf[:, mblk, dblk * P : (dblk + 1) * P],
                                rhs,
                                start=(mblk == 0),
                                stop=(mblk == MB - 1),
                            )
                        nc.any.tensor_copy(out=o_t[:, dblk, :], in_=ps[:])
                    nc.sync.dma_start(
                        out=out[b].rearrange("(dB p) n -> p dB n", p=P)[
                            :, :, c * NT : (c + 1) * NT
                        ],
                        in_=o_t[:],
                    )
    nc.compile()
    return nc


def _emit_transpose(nc, tr_mode, tr_dma, mix_pe_ts, AT, t, e_t, identity, pst_pool, MB):
    use_pe = tr_mode == "pe" or (tr_mode == "mix" and t in mix_pe_ts)
    if not use_pe:
        tr_dma.dma_start_transpose(AT[:, t], e_t[:])
    else:
        # PE transpose in groups of 8 m-blocks per PSUM bank
        # (bf16: 8*128 = 1024 elems = 2 KiB)
        for g in range(MB // 8):
            ps_t = pst_pool.tile([128, 8, 128], BF16)
            for k in range(8):
                mblk = g * 8 + k
                nc.tensor.transpose(
                    ps_t[:, k], e_t[:, mblk * 128 : (mblk + 1) * 128], identity
                )
            nc.any.tensor_copy(out=AT[:, t, g * 8 : (g + 1) * 8, :], in_=ps_t[:])


_nc_cache = {}


def _get_nc(B_local, N, M, D):
    key = (B_local, N, M, D)
    if key not in _nc_cache:
        _nc_cache[key] = build_nc(B_local, N, M, D)
    return _nc_cache[key]


def kernel(u, s):
    u = np.ascontiguousarray(u, dtype=np.float32)
    s = np.ascontiguousarray(s, dtype=np.float32)
    B, N, M = s.shape
    D = u.shape[2]
    assert B % N_CORES == 0
    B_local = B // N_CORES
    nc = _get_nc(B_local, N, M, D)
    in_maps = [
        {
            "s": s[i * B_local : (i + 1) * B_local],
            "u": u[i * B_local : (i + 1) * B_local],
        }
        for i in range(N_CORES)
    ]
    res = run_bass_kernel_spmd(nc, in_maps, core_ids=list(range(N_CORES)))
    return np.concatenate([r["out"] for r in res.results], axis=0)

